# revision 30
# baseline (speedup 1.0000x reference)
"""Trainium2 Bass kernel for nn_BiVision_VQA2 (B=64,T=32,D=768,N=901).

Data-parallel over batch: 8 batch elems per core x 8 cores.
Key math simplifications (validated vs reference in numpy, rel err ~1e-6):
  - ga/go attention use a single key token -> softmax==1 -> those paths are
    linear in cls; question_embeds is mathematically unused.
  - GRU input `a` is constant over time; wx computed once.
  - local attention: scores = (qemb @ W0_h) @ W1_h^T / sqrt(dk) @ X^T ;
    row-constant score terms (K bias, Q.b1) drop out of softmax; query
    pooling applied to the attention matrix before the @X contraction;
    constant bias terms folded into one vector.

Perf structure (rev2):
  - GRU gate math done in transposed "fat" layout [128p, 18*8]: the wh matmul
    stays natural ([8, G] psum, N=512 streams), a DVE add folds wx+biases,
    18 PE transposes flip the summed gates to [G-chunks, batch], and all
    sigmoid/tanh/lerp ops run with 128 lanes instead of 8.
  - fp16 weights/activations on the matmul paths (FWL weight loads, 16-bit
    DVE rate); PSUM accumulation stays fp32.
  - All big weight DMAs (GRU, attention, MLP) are single-descriptor loads
    issued on the gpsimd queue so they stream during the recurrence.
"""

import numpy as np
from contextlib import ExitStack

import concourse.bass as bass
import concourse.tile as tile
from concourse import bacc, mybir
from concourse.bass_utils import run_bass_kernel_spmd
from concourse.masks import make_identity

FP = mybir.dt.float32
FPR = mybir.dt.float32r
OP = mybir.AluOpType
AF = mybir.ActivationFunctionType
BF = mybir.dt.bfloat16
H16 = mybir.dt.float16

NCORES = 8
BL = 8
D = 768
T = 32
G = 3 * D
NK = 900
NH = 2
DK = 384
ET = D // 128
RQ = BL * T
GE = G // 128  # 18 gate chunks
FB = ET * BL   # 48: fat free size per gate third


def chunks(total):
    out, o = [], 0
    while o < total:
        w = min(512, total - o)
        out.append((o, w))
        o += w
    return out


CH_G = chunks(G)
CH_NK = [(0, 512), (512, 388)]
CH_D = [(0, 512), (512, 256)]


def kchunks(n):
    out, o = [], 0
    while o < n:
        out.append((o, min(128, n - o)))
        o += 128
    return out


import os
PHASES = int(os.environ.get("KPHASES", "4"))


def build():
    nc = bacc.Bacc("TRN2", target_bir_lowering=False, debug=False,
                   enable_asserts=False)

    img = nc.dram_tensor("img", [BL, 901, D], FP, kind="ExternalInput").ap()
    h0 = nc.dram_tensor("h0", [BL, D], FP, kind="ExternalInput").ap()
    w_ih = nc.dram_tensor("gru_w_ih", [G, D], FP, kind="ExternalInput").ap()
    w_hh = nc.dram_tensor("gru_w_hh", [G, D], FP, kind="ExternalInput").ap()
    b_ih = nc.dram_tensor("gru_b_ih", [G], FP, kind="ExternalInput").ap()
    b_hh = nc.dram_tensor("gru_b_hh", [G], FP, kind="ExternalInput").ap()
    ga_w = nc.dram_tensor("ga_w", [4, D, D], FP, kind="ExternalInput").ap()
    ga_b = nc.dram_tensor("ga_b", [4, D], FP, kind="ExternalInput").ap()
    ga_pool = nc.dram_tensor("ga_pool", [1], FP, kind="ExternalInput").ap()
    la_w = nc.dram_tensor("la_w", [4, D, D], FP, kind="ExternalInput").ap()
    la_b = nc.dram_tensor("la_b", [4, D], FP, kind="ExternalInput").ap()
    la_pool = nc.dram_tensor("la_pool", [T], FP, kind="ExternalInput").ap()
    go_w = nc.dram_tensor("go_w", [4, D, D], FP, kind="ExternalInput").ap()
    go_b = nc.dram_tensor("go_b", [4, D], FP, kind="ExternalInput").ap()
    go_pool = nc.dram_tensor("go_pool", [T], FP, kind="ExternalInput").ap()
    f1_w = nc.dram_tensor("f1_w", [2 * D, 1024], FP, kind="ExternalInput").ap()
    f1_b = nc.dram_tensor("f1_b", [1024], FP, kind="ExternalInput").ap()
    f2_w = nc.dram_tensor("f2_w", [1024, 512], FP, kind="ExternalInput").ap()
    f2_b = nc.dram_tensor("f2_b", [512], FP, kind="ExternalInput").ap()
    f3_w = nc.dram_tensor("f3_w", [512, 1024], FP, kind="ExternalInput").ap()
    f3_b = nc.dram_tensor("f3_b", [1024], FP, kind="ExternalInput").ap()
    out_d = nc.dram_tensor("out", [BL, 1024], FP, kind="ExternalOutput").ap()

    with tile.TileContext(nc) as tc, ExitStack() as ctx:
        cpool = ctx.enter_context(tc.tile_pool(name="const", bufs=1))
        gstate = ctx.enter_context(tc.tile_pool(name="gstate", bufs=2))
        persist = ctx.enter_context(tc.tile_pool(name="persist", bufs=1))
        psA = ctx.enter_context(tc.tile_pool(name="psA", bufs=3, space="PSUM"))
        psB = ctx.enter_context(tc.tile_pool(name="psB", bufs=2, space="PSUM"))
        psC = ctx.enter_context(tc.tile_pool(name="psC", bufs=1, space="PSUM"))
        psF = ctx.enter_context(tc.tile_pool(name="psF", bufs=1, space="PSUM"))

        ident = cpool.tile([128, 128], FP, tag="ident")
        make_identity(nc, ident[:])
        ones1 = cpool.tile([1, 128], FP, tag="ones1")
        nc.vector.memset(ones1[:], 1.0)
        onesT = cpool.tile([T, 128], FP, tag="onesT")
        nc.vector.memset(onesT[:], 1.0)
        identb = cpool.tile([128, 128], BF, tag="identb")
        nc.vector.tensor_copy(identb[:], ident[:])
        identh = cpool.tile([128, 128], H16, tag="identh")
        nc.vector.tensor_copy(identh[:], ident[:])
        ones1h = cpool.tile([1, 128], H16, tag="ones1h")
        nc.vector.tensor_copy(ones1h[:], ones1[:])

        def colvec(dram_1d, n, tag):
            nt = n // 128
            t_ = cpool.tile([128, nt], FP, tag=tag)
            for j in range(nt):
                nc.sync.dma_start(t_[:, j:j + 1], dram_1d[j * 128:(j + 1) * 128][:, None])
            return t_

        b2gaT = colvec(ga_b[2], D, "b2gaT")
        b3gaT = colvec(ga_b[3], D, "b3gaT")
        b2goT = colvec(go_b[2], D, "b2goT")
        b3goT = colvec(go_b[3], D, "b3goT")
        b0laT = colvec(la_b[0], D, "b0laT")
        b2laT = colvec(la_b[2], D, "b2laT")
        b2laT16 = cpool.tile([128, ET], H16, tag="b2laT16")
        nc.vector.tensor_copy(b2laT16[:], b2laT[:])
        b3laT = colvec(la_b[3], D, "b3laT")
        b1fT = colvec(f1_b, 1024, "b1fT")
        b2fT = colvec(f2_b, 512, "b2fT")
        b3fT = colvec(f3_b, 1024, "b3fT")

        lapool_c = cpool.tile([T, 1], FP, tag="lapool_c")
        nc.sync.dma_start(lapool_c[:], la_pool[:][:, None])
        gopool_c = cpool.tile([T, 1], FP, tag="gopool_c")
        nc.sync.dma_start(gopool_c[:], go_pool[:][:, None])
        gapool_c = cpool.tile([1, 1], FP, tag="gapool_c")
        nc.sync.dma_start(gapool_c[:], ga_pool[:][:, None])

        def sum_bcast(vcol, k, tag):
            p = psC.tile([128, 1], FP, tag="pd")
            lhs = onesT if k == T else ones1
            nc.tensor.matmul(p[:], lhs[:k, :], vcol[:k, :], start=True, stop=True)
            s = cpool.tile([128, 1], FP, tag=tag)
            nc.vector.tensor_copy(s[:], p[:])
            return s

        Sla = sum_bcast(lapool_c, T, "Sla")
        Sgo = sum_bcast(gopool_c, T, "Sgo")
        Sga = sum_bcast(gapool_c, 1, "Sga")

        pmask = cpool.tile([64, 2], FP, tag="pmask")
        nc.vector.memset(pmask[:], 0.0)
        nc.sync.dma_start(pmask[0:T, 0:1], la_pool[:][:, None])
        nc.sync.dma_start(pmask[T:2 * T, 1:2], la_pool[:][:, None])

        # hidden-state history doubles as GRU state: step t writes
        # qembTh[:, t] (contiguous [128, 48]); step t+1 reads it back as the
        # matmul stationary and as h_prev for the lerp.
        qembTh = cpool.tile([128, T, ET, BL], H16, tag="qembTh")
        goutT = cpool.tile([128, ET, BL], H16, tag="goutT")
        aT = cpool.tile([128, ET, BL], H16, tag="aT")
        QtT = persist.tile([128, ET, NH * RQ], BF, tag="QtT")
        # per-batch pooled context, stored already transposed for phase E
        pcxTall = persist.tile([128, ET, NH, BL], H16, tag="pcxTall")
        # phase C/E weights: fp16 tiles; single-descriptor casting DMAs issued
        # on the gpsimd queue right after the GRU weight loads (below), so
        # they stream in while the PE grinds the recurrence.
        f1 = persist.tile([128, 12, 1024], H16, tag="f1")
        f2 = persist.tile([128, 8, 512], H16, tag="f2")
        f3 = persist.tile([128, 4, 1024], H16, tag="f3")
        W3 = persist.tile([128, ET, D], H16, tag="W3")
        W2 = persist.tile([128, ET, D], H16, tag="W2")
        W0h = persist.tile([128, ET, D], H16, tag="W0h")
        W1h = persist.tile([128, ET, D], H16, tag="W1h")

        # ============ phases A + B-pre, overlapped ========================
        # Emission order: build WT(w_ih) first (no input deps — warms the PE
        # while phase A's weight DMAs land), then phase A, then wxb/adder
        # (consume WT-ih + aT), then overwrite WT with w_hh; the build is
        # group-major so GRU step 0 races the tail of the w_hh build.
        with tc.tile_pool(name="wbig", bufs=1) as wbig, \
             tc.tile_pool(name="g1", bufs=1) as g1:
            WT = wbig.tile([128, ET, G], H16, tag="WT")
            adder = g1.tile([BL, G], FP, tag="adder")
            wxnT = g1.tile([128, FB], H16, tag="wxnT")
            hq0 = g1.tile([128, ET, BL], H16, tag="hq0")
            combh = g1.tile([1, G], H16, tag="combh")
            bhhh = g1.tile([1, D], H16, tag="bhhh")

            def build_WT(w_dram, wn):
                nc.gpsimd.dma_start(wn[:], w_dram.rearrange("(c p) d -> p c d", p=128))
                for grp in range(5):
                    gis = list(range(4 * grp, min(4 * grp + 4, GE)))
                    for et in range(ET):
                        pt = psB.tile([128, 512], H16, tag="ptw")
                        for i, gi in enumerate(gis):
                            nc.tensor.matmul(pt[:, 128 * i:128 * (i + 1)],
                                             wn[:, gi, 128 * et:128 * (et + 1)],
                                             identh[:], is_transpose=True,
                                             skip_group_check=True)
                        w0 = 512 * grp
                        wlen = 128 * len(gis)
                        if (et + grp) % 2 == 0:
                            nc.vector.tensor_copy(WT[:, et, w0:w0 + wlen], pt[:, :wlen])
                        else:
                            nc.scalar.copy(WT[:, et, w0:w0 + wlen], pt[:, :wlen])

            with tc.tile_pool(name="wnat1", bufs=1) as wnat1:
                wn1 = wnat1.tile([128, GE, D], H16, tag="wn1")
                build_WT(w_ih, wn1)

            # ---- phase A: cls -> a, gout (overlaps the weight DMAs) ----
            with tc.tile_pool(name="ph0", bufs=1) as ph0:
                clsn = ph0.tile([BL, D], FP, tag="clsn")
                nc.sync.dma_start(clsn[:], img[0:BL, 0, :])
                ptr = psC.tile([128, 512], FP, tag="pd")
                for kt in range(ET):
                    nc.tensor.matmul(ptr[:, 8 * kt:8 * kt + 8], clsn[:, 128 * kt:128 * (kt + 1)],
                                     ident[:BL, :BL], is_transpose=True, skip_group_check=True)
                clsT = ph0.tile([128, ET, BL], H16, tag="clsT")
                nc.vector.tensor_copy(clsT[:].rearrange("p a b -> p (a b)"), ptr[:, :8 * ET])

                def dense_T(w_nat_dram, rhsT, biasT, scaleT, otile, wtag):
                    wsb = ph0.tile([128, ET, D], H16, tag=wtag)
                    nc.gpsimd.dma_start(wsb[:], w_nat_dram.rearrange("(c p) d -> p c d", p=128))
                    for mt in range(ET):
                        p = psC.tile([128, BL], FP, tag="pd")
                        for kt in range(ET):
                            nc.tensor.matmul(p[:], wsb[:, kt, 128 * mt:128 * (mt + 1)],
                                             rhsT[:, kt, :], start=(kt == 0), stop=(kt == ET - 1))
                        if scaleT is None:
                            nc.vector.tensor_scalar(otile[:, mt, :], p[:], biasT[:, mt:mt + 1],
                                                    None, OP.add)
                        else:
                            nc.vector.tensor_scalar(otile[:, mt, :], p[:], biasT[:, mt:mt + 1],
                                                    scaleT[:, 0:1], OP.add, OP.mult)

                A2T = ph0.tile([128, ET, BL], H16, tag="A2T")
                dense_T(ga_w[2], clsT, b2gaT, None, A2T, "wA")
                dense_T(ga_w[3], A2T, b3gaT, Sga, aT, "wB")
                G2T = ph0.tile([128, ET, BL], H16, tag="G2T")
                dense_T(go_w[2], clsT, b2goT, None, G2T, "wA2")
                dense_T(go_w[3], G2T, b3goT, Sgo, goutT, "wB2")

            with tc.tile_pool(name="btmp", bufs=1) as btmp:
                combf = btmp.tile([1, G], FP, tag="combf")
                nc.sync.dma_start(combf[:], b_ih[:][None, :])
                bhhf = btmp.tile([1, G], FP, tag="bhhf")
                nc.sync.dma_start(bhhf[:], b_hh[:][None, :])
                nc.vector.tensor_copy(bhhh[:], bhhf[:, 2 * D:3 * D])
                nc.vector.tensor_add(combf[:, 0:2 * D], combf[:, 0:2 * D], bhhf[:, 0:2 * D])
                nc.vector.tensor_copy(combh[:], combf[:])
                # wxb = a @ W_ih^T + (b_ih + b_hh | b_ih)
                wxb = btmp.tile([BL, G], FP, tag="wxb")
                for (j0, jw) in CH_G:
                    p = psA.tile([BL, 512], FP, tag="whg")
                    for kt in range(ET):
                        nc.tensor.matmul(p[:, :jw], aT[:, kt, :], WT[:, kt, j0:j0 + jw],
                                         start=(kt == 0), stop=False)
                    nc.tensor.matmul(p[:, :jw], ones1h[:1, :BL], combh[:, j0:j0 + jw],
                                     start=False, stop=True)
                    nc.vector.tensor_copy(wxb[:, j0:j0 + jw], p[:, :jw])
                # adder[:, :2D] = wx + b_ih + b_hh (r,z); adder[:, 2D:] = b_hh_n
                nc.vector.tensor_copy(adder[:, 0:2 * D], wxb[:, 0:2 * D])
                for (o, w) in [(0, 512), (512, 256)]:
                    p = psA.tile([BL, 512], FP, tag="whg")
                    nc.tensor.matmul(p[:, :w], ones1h[:1, :BL], bhhh[:, o:o + w],
                                     start=True, stop=True)
                    nc.vector.tensor_copy(adder[:, 2 * D + o:2 * D + o + w], p[:, :w])
                # wxnT = (wx_n + b_ih_n)^T fat [128, 48]
                pwx = psC.tile([128, 512], FP, tag="pd")
                for kt in range(ET):
                    nc.tensor.matmul(pwx[:, 8 * kt:8 * kt + 8],
                                     wxb[:, 2 * D + 128 * kt:2 * D + 128 * (kt + 1)],
                                     ident[:BL, :BL], is_transpose=True, skip_group_check=True)
                nc.vector.tensor_copy(wxnT[:], pwx[:, 0:FB])
                # h0 -> fat fp16 initial state
                hn0 = btmp.tile([BL, D], FP, tag="hn0")
                nc.sync.dma_start(hn0[:], h0[:, :])
                p0 = psC.tile([128, 512], FP, tag="pd")
                for kt in range(ET):
                    nc.tensor.matmul(p0[:, 8 * kt:8 * kt + 8], hn0[:, 128 * kt:128 * (kt + 1)],
                                     ident[:BL, :BL], is_transpose=True, skip_group_check=True)
                nc.scalar.copy(hq0[:].rearrange("p a b -> p (a b)"), p0[:, 0:FB])

            with tc.tile_pool(name="wnat2", bufs=1) as wnat2:
                wn2 = wnat2.tile([128, GE, D], H16, tag="wn2")
                build_WT(w_hh, wn2)

            # all remaining weight loads queue behind the GRU weights on
            # the gpsimd dma queue and stream in during the recurrence.
            nc.gpsimd.dma_start(f1[:], f1_w.rearrange("(c p) d -> p c d", p=128))
            nc.gpsimd.dma_start(f2[:], f2_w.rearrange("(c p) d -> p c d", p=128))
            nc.gpsimd.dma_start(f3[:], f3_w.rearrange("(c p) d -> p c d", p=128))
            nc.gpsimd.dma_start(W3[:], la_w[3].rearrange("(c p) d -> p c d", p=128))
            nc.gpsimd.dma_start(W2[:], la_w[2].rearrange("(c p) d -> p c d", p=128))
            nc.gpsimd.dma_start(W0h[:], la_w[0].rearrange("(c p) d -> p c d", p=128))
            nc.gpsimd.dma_start(W1h[:], la_w[1].rearrange("(c p) d -> p c d", p=128))

            KSTEPS = int(os.environ.get("KSTEPS", str(T)))
            KFILL = int(os.environ.get("KFILL", "3"))
            for t in range(KSTEPS):
                prev = (hq0[:] if t == 0 else qembTh[:, t - 1])
                gs = gstate.tile([BL, G], H16, tag="gs")
                pf = psF.tile([128, GE, BL], H16, tag="fat")
                rzs = g1.tile([128, 2 * FB], H16, tag="rzs")
                for ci in range(5):
                    j0, jw = CH_G[ci]
                    p = psA.tile([BL, 512], FP, tag="whg")
                    for kt in range(ET):
                        stat = (hq0[:, kt, :] if t == 0 else qembTh[:, t - 1, kt, :])
                        nc.tensor.matmul(p[:, :jw], stat, WT[:, kt, j0:j0 + jw],
                                         start=(kt == 0), stop=(kt == ET - 1))
                    nc.vector.tensor_add(gs[:, j0:j0 + jw], p[:, :jw], adder[:, j0:j0 + jw])
                    if ci == 3:
                        for g in range(12):
                            nc.tensor.matmul(pf[:, g, :], gs[:, 128 * g:128 * (g + 1)],
                                             identh[:BL, :BL], is_transpose=True,
                                             skip_group_check=True)
                        nc.scalar.activation(rzs[:], pf[:, 0:12, :].rearrange("p a b -> p (a b)"),
                                             AF.Sigmoid)
                for g in range(12, GE):
                    nc.tensor.matmul(pf[:, g, :], gs[:, 128 * g:128 * (g + 1)],
                                     identh[:BL, :BL], is_transpose=True,
                                     skip_group_check=True)
                if 2 <= t < KSTEPS - 1:
                    for fi in range(KFILL):
                        pdum = psB.tile([128, 512], FP, tag="ptw")
                        nc.tensor.matmul(pdum[:BL, :], hq0[:, fi % ET, :],
                                         WT[:, fi % ET, 0:512], start=True, stop=True)
                # state update in two kt-halves: the next step's matmuls
                # for kt 0..2 depend only on the first half of h_new, so the
                # scheduler can overlap them with the second half.
                t3 = g1.tile([128, FB], H16, tag="t3")
                t4 = g1.tile([128, FB], H16, tag="t4")
                ntf = g1.tile([128, FB], H16, tag="ntf")
                hmn = g1.tile([128, FB], H16, tag="hmn")
                zh2 = g1.tile([128, FB], H16, tag="zh2")
                pfn = pf[:, 12:GE, :].rearrange("p a b -> p (a b)")
                prevf = prev.rearrange("p a b -> p (a b)")
                qdst = qembTh[:, t].rearrange("p a b -> p (a b)")
                HH = FB // 2
                for (ha, hb) in [(0, HH), (HH, FB)]:
                    nc.vector.tensor_mul(t3[:, ha:hb], rzs[:, ha:hb], pfn[:, ha:hb])
                    nc.vector.tensor_add(t4[:, ha:hb], t3[:, ha:hb], wxnT[:, ha:hb])
                    nc.scalar.activation(ntf[:, ha:hb], t4[:, ha:hb], AF.Tanh)
                for (ha, hb) in [(0, HH), (HH, FB)]:
                    nc.vector.tensor_sub(hmn[:, ha:hb], prevf[:, ha:hb], ntf[:, ha:hb])
                    nc.vector.tensor_mul(zh2[:, ha:hb], rzs[:, FB + ha:FB + hb], hmn[:, ha:hb])
                    nc.vector.tensor_add(qdst[:, ha:hb], ntf[:, ha:hb], zh2[:, ha:hb])

        # ================= phase C: Q^T, W1^T, Qt^T =======================
        if PHASES >= 2:
          with tc.tile_pool(name="prep", bufs=1) as prep:
              QT = prep.tile([128, ET, RQ], H16, tag="QT")
              qsrc = qembTh[:].rearrange("p t a b -> p a b t")
              for mt in range(ET):
                  p = psC.tile([128, RQ], FP, tag="pd")
                  for kt in range(ET):
                      nc.tensor.matmul(p[:], W0h[:, kt, 128 * mt:128 * (mt + 1)],
                                       qsrc[:, kt], start=(kt == 0), stop=(kt == ET - 1))
                  nc.vector.tensor_scalar(QT[:, mt, :], p[:], b0laT[:, mt:mt + 1], None, OP.add)
              W1T = prep.tile([128, ET, D], H16, tag="W1T")
              for hd in range(ET):
                  for grp in range(2):
                      pt2 = psB.tile([128, 512], H16, tag="ptw")
                      for i in range(3):
                          e2 = grp * 3 + i
                          nc.tensor.matmul(pt2[:, 128 * i:128 * (i + 1)],
                                           W1h[:, e2, 128 * hd:128 * (hd + 1)],
                                           identh[:], is_transpose=True, skip_group_check=True)
                      if grp == 0:
                          nc.vector.tensor_copy(W1T[:, hd, 0:384], pt2[:, 0:384])
                      else:
                          nc.scalar.copy(W1T[:, hd, 384:768], pt2[:, 0:384])
              scl = 1.0 / float(np.sqrt(DK))
              for h in range(NH):
                  for mt in range(ET):
                      p = psC.tile([128, RQ], FP, tag="pd")
                      for i in range(3):
                          kt = h * 3 + i
                          nc.tensor.matmul(p[:], W1T[:, kt, 128 * mt:128 * (mt + 1)],
                                           QT[:, kt, :], start=(i == 0), stop=(i == 2))
                      dst = QtT[:, mt, :].rearrange("p (b h2 t) -> p b h2 t",
                                                    h2=NH, t=T)[:, :, h, :]
                      nc.scalar.activation(dst, p[:], AF.Copy, scale=scl)

        # ================= phase D: per-b attention =======================
        if PHASES >= 3:
            with tc.tile_pool(name="xb", bufs=2) as xb, \
                 tc.tile_pool(name="ab", bufs=2) as ab:
              KC = kchunks(NK)

              def d_stage1(b):
                  # img load + X transpose: emitted one batch ahead so every
                  # engine queue has ready work while batch b's softmax chain
                  # drains.
                  Xn = xb.tile([128, len(KC), D], BF, tag="Xn")
                  nc.vector.memset(Xn[:, len(KC) - 1, :], 0.0)
                  nc.gpsimd.dma_start(Xn[:, 0:7, :],
                                      img[b, 1:897, :].rearrange("(c p) d -> p c d", p=128))
                  nc.gpsimd.dma_start(Xn[0:4, 7, :], img[b, 897:901, :])
                  XT = xb.tile([128, ET, len(KC) * 128], BF, tag="XT")
                  for et in range(ET):
                      for g in range(2):
                          pt = psB.tile([128, 512], BF, tag="ptw")
                          for i in range(4):
                              c = g * 4 + i
                              nc.tensor.matmul(pt[:, 128 * i:128 * (i + 1)],
                                               Xn[:, c, 128 * et:128 * (et + 1)],
                                               identb[:], is_transpose=True,
                                               skip_group_check=True)
                          if (et + g) % 2 == 0:
                              nc.vector.tensor_copy(XT[:, et, 512 * g:512 * (g + 1)], pt[:])
                          else:
                              nc.scalar.copy(XT[:, et, 512 * g:512 * (g + 1)], pt[:])
                  return Xn, XT

              cur = d_stage1(0)
              for b in range(BL):
                  Xn, XT = cur
                  if b + 1 < BL:
                      cur = d_stage1(b + 1)
                  att = ab.tile([64, NK], BF, tag="att")
                  zacc = ab.tile([64, 2], FP, tag="zacc")
                  for ci, (n0, nw) in enumerate(CH_NK):
                      p = psA.tile([64, 512], FP, tag="whg")
                      for kt in range(ET):
                          nc.tensor.matmul(p[:, :nw],
                                           QtT[:, kt, b * 2 * T:(b + 1) * 2 * T],
                                           XT[:, kt, n0:n0 + nw],
                                           start=(kt == 0), stop=(kt == ET - 1))
                      nc.scalar.activation(att[:, n0:n0 + nw], p[:, :nw], AF.Exp,
                                           accum_out=zacc[:, ci:ci + 1])
                  zs = ab.tile([64, 1], FP, tag="zs")
                  nc.vector.tensor_add(zs[:], zacc[:, 0:1], zacc[:, 1:2])
                  rz = ab.tile([64, 1], FP, tag="rz1")
                  nc.vector.reciprocal(rz[:], zs[:])
                  wm = ab.tile([64, 2], BF, tag="wm")
                  nc.vector.tensor_scalar(wm[:], pmask[:], rz[:, 0:1], None, OP.mult)
                  # paT[k, h] = sum_q att[q, k] * wm[q, h]: att chunk as the
                  # stationary gives pa already transposed, no extra transpose
                  paT = ab.tile([128, len(KC), 2], BF, tag="paT")
                  nc.vector.memset(paT[:].rearrange("p a b -> p (a b)"), 0.0)
                  ptp = psC.tile([128, 512], FP, tag="pd2")
                  for c, (k0, kw) in enumerate(KC):
                      nc.tensor.matmul(ptp[:kw, 2 * c:2 * c + 2], att[:, k0:k0 + kw],
                                       wm[:], start=True, stop=True, skip_group_check=True)
                  nc.vector.tensor_copy(paT[:, 0:7, :].rearrange("p a b -> p (a b)"),
                                        ptp[:, 0:14])
                  nc.vector.tensor_copy(paT[:4, 7, :], ptp[:4, 14:16])
                  pcx_sb = ab.tile([2, D], FP, tag="pcx_sb")
                  for ci, (n0, nw) in enumerate(CH_D):
                      p = psA.tile([64, 512], FP, tag="whg")
                      for c in range(len(KC)):
                          nc.tensor.matmul(p[:2, :nw], paT[:, c, :],
                                           Xn[:, c, n0:n0 + nw],
                                           start=(c == 0), stop=(c == len(KC) - 1))
                      nc.vector.tensor_copy(pcx_sb[:, n0:n0 + nw], p[:2, :nw])
                  # transpose pooled ctx into persistent [128, ET, 2, b] slot
                  ptq = psC.tile([128, 512], FP, tag="pd")
                  for kt in range(ET):
                      nc.tensor.matmul(ptq[:, 2 * kt:2 * kt + 2],
                                       pcx_sb[:, 128 * kt:128 * (kt + 1)],
                                       ident[:2, :2], is_transpose=True, skip_group_check=True)
                  for kt in range(ET):
                      nc.vector.tensor_copy(pcxTall[:, kt, :, b], ptq[:, 2 * kt:2 * kt + 2])

        # ================= phase E: projections + MLP =====================
        if PHASES >= 4:
            with tc.tile_pool(name="tail", bufs=1) as tail:
              vconT = tail.tile([128, ET], FP, tag="vconT")
              for mt in range(ET):
                  p = psC.tile([128, 1], FP, tag="pd")
                  for kt in range(ET):
                      nc.tensor.matmul(p[:], W3[:, kt, 128 * mt:128 * (mt + 1)],
                                       b2laT16[:, kt:kt + 1], start=(kt == 0), stop=(kt == ET - 1))
                  nc.vector.tensor_scalar(vconT[:, mt:mt + 1], p[:], b3laT[:, mt:mt + 1],
                                          Sla[:, 0:1], OP.add, OP.mult)
              pctxT = tail.tile([128, ET, BL], H16, tag="pctxT")
              for h in range(NH):
                  for mi in range(3):
                      mt = h * 3 + mi
                      p = psC.tile([128, BL], FP, tag="pd")
                      for kt in range(ET):
                          nc.tensor.matmul(p[:], W2[:, kt, 128 * mt:128 * (mt + 1)],
                                           pcxTall[:, kt, h, :], start=(kt == 0), stop=(kt == ET - 1))
                      nc.vector.tensor_copy(pctxT[:, mt, :], p[:])
              loT = tail.tile([128, ET, BL], H16, tag="loT")
              for mt in range(ET):
                  p = psC.tile([128, BL], FP, tag="pd")
                  for kt in range(ET):
                      nc.tensor.matmul(p[:], W3[:, kt, 128 * mt:128 * (mt + 1)],
                                       pctxT[:, kt, :], start=(kt == 0), stop=(kt == ET - 1))
                  nc.vector.tensor_scalar(loT[:, mt, :], p[:], vconT[:, mt:mt + 1], None, OP.add)

              y1T = tail.tile([128, 8, BL], H16, tag="y1T")
              for mt in range(8):
                  p = psC.tile([128, BL], FP, tag="pd")
                  for kt in range(12):
                      r_ = loT[:, kt, :] if kt < ET else goutT[:, kt - ET, :]
                      nc.tensor.matmul(p[:], f1[:, kt, 128 * mt:128 * (mt + 1)], r_,
                                       start=(kt == 0), stop=(kt == 11))
                  nc.vector.tensor_scalar(y1T[:, mt, :], p[:], b1fT[:, mt:mt + 1], None, OP.add)
              y2T = tail.tile([128, 4, BL], H16, tag="y2T")
              for mt in range(4):
                  p = psC.tile([128, BL], FP, tag="pd")
                  for kt in range(8):
                      nc.tensor.matmul(p[:], f2[:, kt, 128 * mt:128 * (mt + 1)],
                                       y1T[:, kt, :], start=(kt == 0), stop=(kt == 7))
                  nc.scalar.activation(y2T[:, mt, :], p[:], AF.Relu, bias=b2fT[:, mt:mt + 1])
              yT = tail.tile([128, 8, BL], FP, tag="yT")
              for mt in range(8):
                  p = psC.tile([128, BL], FP, tag="pd")
                  for kt in range(4):
                      nc.tensor.matmul(p[:], f3[:, kt, 128 * mt:128 * (mt + 1)],
                                       y2T[:, kt, :], start=(kt == 0), stop=(kt == 3))
                  nc.vector.tensor_scalar(yT[:, mt, :], p[:], b3fT[:, mt:mt + 1], None, OP.add)
              ynat = tail.tile([BL, 1024], FP, tag="ynat")
              for g in range(2):
                  po = psB.tile([128, 512], FP, tag="ptw")
                  for i in range(4):
                      mt = g * 4 + i
                      nc.tensor.matmul(po[:BL, 128 * i:128 * (i + 1)], yT[:, mt, :],
                                       ident[:128, :128], is_transpose=True,
                                       skip_group_check=True)
                  nc.vector.tensor_copy(ynat[:, 512 * g:512 * (g + 1)], po[:BL, :])
              nc.sync.dma_start(out_d[:, :], ynat[:])

    nc.compile()
    return nc


_NC = None


def kernel(**inputs):
    global _NC
    if _NC is None:
        _NC = build()
    B = inputs["image_local_embeds"].shape[0]
    per = B // NCORES
    in_maps = []
    for c in range(NCORES):
        sl = slice(c * per, (c + 1) * per)
        m = {
            "img": np.ascontiguousarray(np.asarray(inputs["image_local_embeds"])[sl], dtype=np.float32),
            "h0": np.ascontiguousarray(np.asarray(inputs["h0"])[sl], dtype=np.float32),
        }
        for k in ["gru_w_ih", "gru_w_hh", "gru_b_ih", "gru_b_hh", "ga_w", "ga_b",
                  "ga_pool", "la_w", "la_b", "la_pool", "go_w", "go_b", "go_pool",
                  "f1_w", "f1_b", "f2_w", "f2_b", "f3_w", "f3_b"]:
            m[k] = np.ascontiguousarray(np.asarray(inputs[k], dtype=np.float32))
        in_maps.append(m)
    res = run_bass_kernel_spmd(_NC, in_maps, core_ids=list(range(NCORES)))
    return np.concatenate([res.results[c]["out"] for c in range(NCORES)], axis=0)
